# revision 23
# baseline (speedup 1.0000x reference)
"""Multi-head attention (B=2, S=2048, D=1024, H=16, dh=64) on 8 TRN2 NeuronCores.

Sharding: data-parallel over batch (2) x tensor-parallel over heads (4 per core).
Core c handles batch c//4 and heads [4*(c%4), 4*(c%4)+4). Each core computes a
partial output (its heads' contribution through Wo); the host sums the 4 partials
per batch and adds bo (the unshard step for a sum-sharded tensor).

The schedule is built around the Activation engine: exp over the full S x S x 4
logits is ~137us of ACT time, more than the PE's ~109us of attention matmuls, so
ACT must start early and never starve. To that end:
  - Host supplies activations in chunk-contiguous layout ([NCH, P, KO, CH], 8KB
    per-partition descriptors) and DMAs are emitted in consumption order, so the
    K projection starts ~5us in.
  - QK for query-chunk 0 is staged per KEY chunk: logits for key tiles of chunk
    kc are computed (and exp'd) right after the K projection of chunk kc, so the
    first EXP issues at ~12us instead of after the full K projection.
  - V/Q projections, the next chunk's QK, and the output projection of the
    previous chunk are interleaved into the attention loop, filling the PE while
    ACT paces, and eliminating the output-projection tail.
  - Softmax normalization uses reciprocal_approx_fast (single DVE op, ~5x faster
    than the exact iterative divide; denominators are sums of positive exps so
    the approx domain is safe) and multiplies straight out of PSUM.

Matmuls run in bf16 with f32 PSUM accumulation. The PV matmul carries an extra
ones-column in the stationary operand so the softmax denominator falls out of
the same accumulation for free; bv is pre-added to V (P @ (V + 1*bv) = PV +
denom*bv, so the post-divide result already includes bv). Every intermediate is
produced in the layout its consumer wants, so there are no on-device transposes.
"""

import sys

if "/opt/trn_rl_repo" not in sys.path:
    sys.path.insert(0, "/opt/trn_rl_repo")

import ml_dtypes
import numpy as np

import concourse.bass as bass
import concourse.mybir as mybir
import concourse.tile as tile
from concourse import bacc, bass_utils
from concourse.bass import ts

# Problem constants (hardcoded per contract)
B, S, D = 2, 2048, 1024
H, DH = 16, 64            # total heads, head dim
HC = 4                    # heads per core
DHC = HC * DH             # 256 projected dims per core
NCORES = 8
P = 128
CH = 512                  # query-chunk for attention / projection sub-chunk
NCH = S // CH             # 4
TT = S // P               # 16 key tiles
KO = D // P               # 8 contraction tiles for projections

f32 = mybir.dt.float32
bf16 = mybir.dt.bfloat16
EXP = mybir.ActivationFunctionType.Exp

_compiled = None          # cached nc across calls
last_results = None       # BassKernelResults of the most recent run (for profiling)


def _build():
    nc = bacc.Bacc("TRN2", target_bir_lowering=False, debug=False)

    # Per-core DRAM parameters. Activations pre-transposed AND pre-chunked on
    # host: x[c, p, ko, s] = x^T[ko*128+p, c*512+s], so each chunk is 8KB
    # contiguous per partition (fat DMA descriptors).
    qT = nc.dram_tensor("qT", [NCH, P, KO, CH], bf16, kind="ExternalInput")
    kT = nc.dram_tensor("kT", [NCH, P, KO, CH], bf16, kind="ExternalInput")
    vT = nc.dram_tensor("vT", [NCH, P, KO, CH], bf16, kind="ExternalInput")
    # Weights pre-arranged to [P, KO, .] on host (4KB/partition descriptors).
    wq = nc.dram_tensor("wq", [P, KO, DHC], bf16, kind="ExternalInput")
    wk = nc.dram_tensor("wk", [P, KO, DHC], bf16, kind="ExternalInput")
    wv = nc.dram_tensor("wv", [P, KO, DHC], bf16, kind="ExternalInput")
    wo = nc.dram_tensor("wo", [P, DHC // P, D], bf16, kind="ExternalInput")
    bq = nc.dram_tensor("bq", [DHC], f32, kind="ExternalInput")
    bk = nc.dram_tensor("bk", [DHC], f32, kind="ExternalInput")
    bv = nc.dram_tensor("bv", [DHC], f32, kind="ExternalInput")
    out = nc.dram_tensor("out", [S, D], bf16, kind="ExternalOutput")

    with tile.TileContext(nc) as tc:
        with (
            tc.tile_pool(name="weights", bufs=1) as wpool,
            tc.tile_pool(name="acts", bufs=1) as apool,
            tc.tile_pool(name="xin", bufs=7) as xpool,
            tc.tile_pool(name="pt", bufs=5) as ptpool,
            tc.tile_pool(name="small", bufs=2) as spool,
            tc.tile_pool(name="outs", bufs=2) as opool,
            tc.tile_pool(name="io_ps", bufs=2, space="PSUM") as io_ps,
            tc.tile_pool(name="l_ps", bufs=3, space="PSUM") as l_ps,
        ):
            # ---- persistent SBUF tiles ----
            wq_sb = wpool.tile([P, KO, DHC], bf16, tag="wq")
            wk_sb = wpool.tile([P, KO, DHC], bf16, tag="wk")
            wv_sb = wpool.tile([P, KO, DHC], bf16, tag="wv")
            wo_sb = wpool.tile([P, DHC // P, D], bf16, tag="wo")
            bq_sb = wpool.tile([P, 2], f32, tag="bq")
            bk_sb = wpool.tile([P, 2], f32, tag="bk")
            bv_row = wpool.tile([P, DHC], f32, tag="bv_row")
            bv_bc = wpool.tile([P, DHC], f32, tag="bv_bc")
            bv_heads = bv_bc[:, :].rearrange("p (h c) -> p h c", c=DH)
            warm = wpool.tile([P, 2], f32, tag="warm")

            # q^T/k^T: [P, m, S] where projected dim r lives at (r % 128, r // 128)
            q_sb = apool.tile([P, 2, S], bf16, tag="q")
            k_sb = apool.tile([P, 2, S], bf16, tag="k")
            # v natural + ones column per head (65-strided), padded so every
            # head's stationary slice can be 128 columns wide.
            VW = HC * (DH + 1)
            v_sb = apool.tile([P, TT, VW + P - (DH + 1)], bf16, tag="v")
            v_heads = v_sb[:, :, 0:VW].rearrange("p tt (h c) -> p tt h c", c=DH + 1)
            attn_sb = apool.tile([P, 2, S], bf16, tag="attn")
            ones_f32 = wpool.tile([P, TT, HC], f32, tag="ones")

            # ---- cheap setup first: the warmup EXP's const-bias DMA must hit
            # the SP queue before the big input transfers, so the ACT table
            # load happens at t~1us instead of behind 11MB of DMA.
            nc.vector.memset(ones_f32, 1.0)
            nc.scalar.activation(out=warm[0:1, :], in_=ones_f32[0:1, 0, 0:2], func=EXP)
            nc.vector.tensor_copy(out=v_heads[:, :, :, DH], in_=ones_f32)
            nc.vector.memset(v_sb[:, :, VW:], 0.0)

            # ---- DMA emission in consumption-priority order. Each dma_start
            # lands on one ~26GB/s hardware queue, so latency-critical
            # transfers are split into pieces to engage several queues.
            # Wave 1: the critical 3MB (wk, k-chunk 0, wq, q-chunk 0) split
            # into 128KB pieces issued from BOTH HWDGE engines (SP and ACT)
            # concurrently -- each dma_start is a ~600ns sequencer
            # instruction, so a single engine can only launch ~1.7 pieces/us.
            # ACT is idle until the first exp (~17us) and its triggers all
            # fire well before that. Non-critical tensors follow, ordered by
            # consumption time.
            def dma_range(dst, src, k0, k1, step, eng=None):
                eng = eng or nc.sync
                for i in range(k0, k1, step):
                    ksl = slice(i, i + step)
                    eng.dma_start(out=dst[:, ksl], in_=src[:, ksl])

            xk = {}
            xq = {}
            xv = {}
            for c in range(NCH):
                xk[c] = xpool.tile([P, KO, CH], bf16, tag="x", name=f"xk{c}")
                xq[c] = xpool.tile([P, KO, CH], bf16, tag="x", name=f"xq{c}")
                if c > 1:
                    break
            # ACT: biases first (tiny), then wq + q-chunk 0
            nc.scalar.dma_start(out=bk_sb, in_=bk.ap().rearrange("(mo p) -> p mo", p=P))
            nc.scalar.dma_start(out=bq_sb, in_=bq.ap().rearrange("(mo p) -> p mo", p=P))
            nc.scalar.dma_start(out=bv_row[0:1, :], in_=bv.ap().rearrange("(a d) -> a d", a=1))
            dma_range(wq_sb, wq.ap(), 0, 8, 2, eng=nc.scalar)
            dma_range(xq[0], qT.ap()[0], 0, 8, 1, eng=nc.scalar)
            # SP: wk + k-chunk 0
            dma_range(wk_sb, wk.ap(), 0, 8, 2)
            dma_range(xk[0], kT.ap()[0], 0, 8, 1)
            # non-critical, by consumption order; split across both engines
            dma_range(xk[1], kT.ap()[1], 0, 8, 4)
            xk[2] = xpool.tile([P, KO, CH], bf16, tag="x", name="xk2")
            dma_range(xk[2], kT.ap()[2], 0, 8, 4)
            xk[3] = xpool.tile([P, KO, CH], bf16, tag="x", name="xk3")
            dma_range(xk[3], kT.ap()[3], 0, 8, 4)
            dma_range(xq[1], qT.ap()[1], 0, 8, 4)
            xq[2] = xpool.tile([P, KO, CH], bf16, tag="x", name="xq2")
            dma_range(xq[2], qT.ap()[2], 0, 8, 4, eng=nc.scalar)
            xq[3] = xpool.tile([P, KO, CH], bf16, tag="x", name="xq3")
            dma_range(xq[3], qT.ap()[3], 0, 8, 4, eng=nc.scalar)
            dma_range(wv_sb, wv.ap(), 0, 8, 4, eng=nc.scalar)
            xv[0] = xpool.tile([P, KO, CH], bf16, tag="x", name="xv0")
            dma_range(xv[0], vT.ap()[0], 0, 8, 4, eng=nc.scalar)
            xv[1] = xpool.tile([P, KO, CH], bf16, tag="x", name="xv1")
            dma_range(xv[1], vT.ap()[1], 0, 8, 4)
            xv[2] = xpool.tile([P, KO, CH], bf16, tag="x", name="xv2")
            dma_range(xv[2], vT.ap()[2], 0, 8, 4)
            xv[3] = xpool.tile([P, KO, CH], bf16, tag="x", name="xv3")
            dma_range(xv[3], vT.ap()[3], 0, 8, 4)
            nc.sync.dma_start(out=wo_sb[:, 0], in_=wo.ap()[:, 0])
            nc.sync.dma_start(out=wo_sb[:, 1], in_=wo.ap()[:, 1])

            nc.gpsimd.partition_broadcast(bv_bc, bv_row[0:1, :])

            # ---- emission helpers ----
            # Projections are emitted through a FIFO of small "filler pieces"
            # (2-4 matmuls each) interleaved between QK tiles: the exp stream
            # paces QK via the l_ps pool at ~1.06us/tile while a QK tile is
            # only ~0.43us of PE work, so each tile slot calls fill(2) to top
            # the PE up and keep its clock from dropping. A projection's PSUM
            # accumulation stays open across pieces (different banks than QK).
            fill_q = []
            _fseq = [0]

            def _fname():
                _fseq[0] += 1
                return f"fps{_fseq[0]}"

            def add_kq(c, w_sb, b_sb, xd, dst, m):
                sl = slice(c * CH, (c + 1) * CH)
                state = {}

                def piece(k0):
                    def f():
                        if "ps" not in state:
                            state["ps"] = io_ps.tile([P, CH], f32, tag="ps",
                                                     name=_fname())
                        ps = state["ps"]
                        for ko in range(k0, k0 + 2):
                            nc.tensor.matmul(ps, w_sb[:, ko, ts(m, P)],
                                             xd[:, ko, :],
                                             start=(ko == 0), stop=(ko == KO - 1))
                        if k0 + 2 == KO:
                            nc.vector.tensor_scalar_add(
                                out=dst[:, m, sl], in0=ps,
                                scalar1=b_sb[:, m : m + 1])
                    return f

                for k0 in range(0, KO, 2):
                    fill_q.append(piece(k0))

            def add_kp(c, m):
                add_kq(c, wk_sb, bk_sb, xk[c], k_sb, m)

            def add_qp(c, m):
                add_kq(c, wq_sb, bq_sb, xq[c], q_sb, m)

            def add_vp(c, th):
                tt = (c * CH) // P + th
                state = {}

                def piece(k0):
                    def f():
                        if "ps" not in state:
                            state["ps"] = io_ps.tile([P, CH], f32, tag="ps",
                                                     name=_fname())
                        ps = state["ps"]
                        for ko in range(k0, k0 + 4):
                            nc.tensor.matmul(ps[:, 0:DHC],
                                             xv[c][:, ko, ts(th, P)],
                                             wv_sb[:, ko, :],
                                             start=(ko == 0), stop=(ko == KO - 1))
                        if k0 + 4 == KO:
                            nc.vector.tensor_add(
                                out=v_heads[:, tt, :, 0:DH],
                                in0=ps[:, 0:DHC].rearrange("p (h c) -> p h c",
                                                           c=DH),
                                in1=bv_heads,
                            )
                    return f

                for k0 in range(0, KO, 4):
                    fill_q.append(piece(k0))

            def fill(n):
                for _ in range(n):
                    if fill_q:
                        fill_q.pop(0)()

            pts = {}  # (c, h) -> exp'd logits [P, TT, CH], keys on partitions

            def qk_head(c, h, tbs):
                """QK + exp for query-chunk c, head h, tb pairs in tbs."""
                csl = slice(c * CH, (c + 1) * CH)
                if (c, h) not in pts:
                    pts[(c, h)] = ptpool.tile([P, TT, CH], bf16, tag="pt",
                                              name=f"pt_c{c}_h{h}")
                base = DH * (h % 2)
                m = h // 2
                for tb in tbs:
                    ps = l_ps.tile([P, 2, CH], f32, tag="l")
                    for j in range(2):
                        tt = 2 * tb + j
                        nc.tensor.matmul(
                            ps[:, j, :],
                            k_sb[base : base + DH, m, ts(tt, P)],
                            q_sb[base : base + DH, m, csl],
                            start=True, stop=True,
                        )
                    nc.scalar.activation(out=pts[(c, h)][:, 2 * tb : 2 * tb + 2, :],
                                         in_=ps, func=EXP)

            def pv_head(c, h):
                """PV + normalize for query-chunk c, head h."""
                csl = slice(c * CH, (c + 1) * CH)
                base = DH * (h % 2)
                m = h // 2
                po = io_ps.tile([P, CH], f32, tag="ps")
                for tt in range(TT):
                    nc.tensor.matmul(
                        po[0 : DH + 1, :],
                        v_heads[:, tt, h, :],
                        pts[(c, h)][:, tt, :],
                        start=(tt == 0), stop=(tt == TT - 1),
                    )
                # approx reciprocal must read SBUF at base partition 0 (the
                # custom-DVE op mis-addresses PSUM rows at nonzero bases)
                rin = spool.tile([P, CH], f32, tag="rin")
                nc.vector.tensor_copy(out=rin[0:1, :], in_=po[DH : DH + 1, :])
                rec = spool.tile([P, CH], f32, tag="rec")
                nc.vector.reciprocal_approx_fast(out=rec[0:1, :], in_=rin[0:1, :])
                bc = spool.tile([P, CH], f32, tag="bc")
                nc.gpsimd.partition_broadcast(bc[0:DH, :], rec[0:1, :])
                nc.vector.tensor_mul(
                    out=attn_sb[base : base + DH, m, csl],
                    in0=po[0:DH, :], in1=bc[0:DH, :],
                )

            def out_proj(c):
                # Last chunk: evacuate PSUM on the (by then idle) ACT engine so
                # the PE doesn't stall on DVE evacs in the tail; split each
                # store across two DMA queues to shrink the drain.
                last = c == NCH - 1
                for th in range(CH // P):
                    st = (c * CH) // P + th
                    for n in range(2):
                        pw = io_ps.tile([P, CH], f32, tag="ps")
                        for ko in range(2):
                            nc.tensor.matmul(pw, attn_sb[:, ko, ts(st, P)],
                                             wo_sb[:, ko, ts(n, 512)],
                                             start=(ko == 0), stop=(ko == 1))
                        ot = opool.tile([P, CH], bf16, tag="ot")
                        if last and n == 0:
                            nc.scalar.copy(out=ot, in_=pw)
                        else:
                            nc.vector.tensor_copy(out=ot, in_=pw)
                        for half in range(2):
                            hs = slice(half * 256, (half + 1) * 256)
                            nc.sync.dma_start(
                                out=out.ap()[ts(st, P), n * 512 + half * 256 :
                                             n * 512 + (half + 1) * 256],
                                in_=ot[:, hs])

            # ---- phase A: chunk-0 QK staged by key chunk at l_ps-tile
            # granularity, with projection work as filler between QK tiles so
            # the in-order PE queue never runs dry while the exp stream paces.
            # Stage kc's QK tiles read K tiles of k-chunk kc, so kp(kc) must
            # be fully emitted before that stage; Vp/Qp filler is ordered to
            # match DMA arrival.
            def qk1(h, tb):
                qk_head(0, h, (tb,))

            def kq_direct(c, w_sb, b_sb, xd, dst, m):
                sl = slice(c * CH, (c + 1) * CH)
                ps = io_ps.tile([P, CH], f32, tag="ps")
                for ko in range(KO):
                    nc.tensor.matmul(ps, w_sb[:, ko, ts(m, P)], xd[:, ko, :],
                                     start=(ko == 0), stop=(ko == KO - 1))
                nc.vector.tensor_scalar_add(out=dst[:, m, sl], in0=ps,
                                            scalar1=b_sb[:, m : m + 1])

            # critical path straight-line: K/Q chunk-0 m0 projections
            kq_direct(0, wk_sb, bk_sb, xk[0], k_sb, 0)
            kq_direct(0, wq_sb, bq_sb, xq[0], q_sb, 0)
            # filler FIFO in dependency/DMA-arrival order
            add_kp(0, 1); add_qp(0, 1)
            add_kp(1, 0); add_kp(1, 1)
            add_qp(1, 0); add_qp(1, 1)
            add_kp(2, 0); add_kp(2, 1)
            add_kp(3, 0); add_kp(3, 1)
            add_qp(2, 0); add_qp(2, 1)
            add_qp(3, 0); add_qp(3, 1)
            for c in range(NCH):
                for th in range(CH // P):
                    add_vp(c, th)
            # stage 0: m0 heads first (their inputs are ready), m1 heads after
            # the kp/qp(0,1) fillers have drained
            for tb in (0, 1):
                qk1(0, tb); fill(2)
                qk1(1, tb); fill(2)
            for tb in (0, 1):
                qk1(2, tb); fill(2)
                qk1(3, tb); fill(2)
            # stages 1-3
            for tb in range(2, 8):
                for h in range(HC):
                    qk1(h, tb); fill(2)
            for tb in range(8):                # 1-chunk QK lookahead (h0)
                qk_head(1, 0, (tb,))
                fill(2)
            while fill_q:                      # drain: all V must be resident
                fill(1)                        # before the first PV below

            # ---- steady state ----
            # Per chunk c: blocks of [PV(c,h) + normalize, next chunk's QK
            # tiles, out-proj groups of chunk c-1]. The QK tiles are paced by
            # the exp stream via the l_ps pool; PV and out groups sit between
            # them so the PE's 4-deep lookahead always has runnable work and
            # the clock never drops. out(c3) runs as a tail burst with ACT
            # evacuation (ACT is idle by then).
            def out_group(c, g):
                st = (c * CH) // P + g // 2
                n = g % 2
                last = c == NCH - 1
                pw = io_ps.tile([P, CH], f32, tag="ps")
                for ko in range(2):
                    nc.tensor.matmul(pw, attn_sb[:, ko, ts(st, P)],
                                     wo_sb[:, ko, ts(n, 512)],
                                     start=(ko == 0), stop=(ko == 1))
                ot = opool.tile([P, CH], bf16, tag="ot")
                if last:
                    nc.scalar.copy(out=ot, in_=pw)
                else:
                    nc.vector.tensor_copy(out=ot, in_=pw)
                for half in range(2):
                    nc.sync.dma_start(
                        out=out.ap()[ts(st, P), n * 512 + half * 256 :
                                     n * 512 + (half + 1) * 256],
                        in_=ot[:, half * 256 : (half + 1) * 256])

            for c in range(NCH):
                if c == NCH - 1:
                    # last iter: flush out(c2) first so the PV(c3) blocks and
                    # their normalize chains pipeline without io-ring stalls
                    for g in range(8):
                        out_group(c - 1, g)
                for h in range(HC):
                    pv_head(c, h)
                    qh = h + 1 if c == 0 else h  # c0 continues at (c1, h1)
                    tiles = list(range(8)) if c + 1 < NCH and qh < HC else []
                    for i, tb in enumerate(tiles):
                        qk_head(c + 1, qh, (tb,))
                        if 0 < c < NCH - 1 and i in (1, 4):
                            out_group(c - 1, 2 * h + (0 if i == 1 else 1))
            out_proj(NCH - 1)

    nc.finalize()
    return nc


def kernel(**inputs):
    global _compiled, last_results
    if _compiled is None:
        _compiled = _build()
    nc = _compiled

    query = np.asarray(inputs["query"], np.float32)
    key = np.asarray(inputs["key"], np.float32)
    value = np.asarray(inputs["value"], np.float32)
    Wq = np.asarray(inputs["Wq"], np.float32)
    Wk = np.asarray(inputs["Wk"], np.float32)
    Wv = np.asarray(inputs["Wv"], np.float32)
    Wo = np.asarray(inputs["Wo"], np.float32)
    bq_f = np.asarray(inputs["bq"], np.float32)
    bk_f = np.asarray(inputs["bk"], np.float32)
    bv_f = np.asarray(inputs["bv"], np.float32)
    bo_f = np.asarray(inputs["bo"], np.float32)

    bf = ml_dtypes.bfloat16
    scale = 1.0 / np.sqrt(np.float32(DH))

    def chunked(x):  # [S, D] -> [NCH, P, KO, CH] with x^T chunk-contiguous
        xt = np.ascontiguousarray(x.T)                       # [D, S]
        return np.ascontiguousarray(
            xt.reshape(KO, P, NCH, CH).transpose(2, 1, 0, 3)
        ).astype(bf)

    def wlayout(w):  # [D, DHC] -> [P, KO, DHC]
        return np.ascontiguousarray(w.reshape(KO, P, DHC).transpose(1, 0, 2)).astype(bf)

    qTc = [chunked(query[b]) for b in range(B)]
    kTc = [chunked(key[b]) for b in range(B)]
    vTc = [chunked(value[b]) for b in range(B)]

    in_maps = []
    for c in range(NCORES):
        b = c // 4
        sh = c % 4
        sl = slice(DHC * sh, DHC * (sh + 1))
        in_maps.append({
            "qT": qTc[b], "kT": kTc[b], "vT": vTc[b],
            "wq": wlayout(Wq[:, sl] * scale),
            "wk": wlayout(Wk[:, sl]),
            "wv": wlayout(Wv[:, sl]),
            "wo": np.ascontiguousarray(
                Wo[sl, :].reshape(DHC // P, P, D).transpose(1, 0, 2)
            ).astype(bf),
            "bq": np.ascontiguousarray(bq_f[sl]) * scale,
            "bk": np.ascontiguousarray(bk_f[sl]),
            "bv": np.ascontiguousarray(bv_f[sl]),
        })

    res = bass_utils.run_bass_kernel_spmd(nc, in_maps, core_ids=list(range(NCORES)))
    last_results = res

    final = np.empty((B, S, D), np.float32)
    for b in range(B):
        acc = res.results[4 * b]["out"].astype(np.float32)
        for sh in range(1, 4):
            acc = acc + res.results[4 * b + sh]["out"].astype(np.float32)
        final[b] = acc + bo_f
    return final


# revision 24
# speedup vs baseline: 1.0479x; 1.0479x over previous
"""Multi-head attention (B=2, S=2048, D=1024, H=16, dh=64) on 8 TRN2 NeuronCores.

Sharding: data-parallel over batch (2) x tensor-parallel over heads (4 per core).
Core c handles batch c//4 and heads [4*(c%4), 4*(c%4)+4). Each core computes a
partial output (its heads' contribution through Wo); the host sums the 4 partials
per batch and adds bo (the unshard step for a sum-sharded tensor).

The schedule is built around the Activation engine: exp over the full S x S x 4
logits is ~137us of ACT time, more than the PE's ~109us of attention matmuls, so
ACT must start early and never starve. To that end:
  - Host supplies activations in chunk-contiguous layout ([NCH, P, KO, CH], 8KB
    per-partition descriptors) and DMAs are emitted in consumption order, so the
    K projection starts ~5us in.
  - QK for query-chunk 0 is staged per KEY chunk: logits for key tiles of chunk
    kc are computed (and exp'd) right after the K projection of chunk kc, so the
    first EXP issues at ~12us instead of after the full K projection.
  - V/Q projections, the next chunk's QK, and the output projection of the
    previous chunk are interleaved into the attention loop, filling the PE while
    ACT paces, and eliminating the output-projection tail.
  - Softmax normalization uses reciprocal_approx_fast (single DVE op, ~5x faster
    than the exact iterative divide; denominators are sums of positive exps so
    the approx domain is safe) and multiplies straight out of PSUM.

Matmuls run in bf16 with f32 PSUM accumulation. The PV matmul carries an extra
ones-column in the stationary operand so the softmax denominator falls out of
the same accumulation for free; bv is pre-added to V (P @ (V + 1*bv) = PV +
denom*bv, so the post-divide result already includes bv). Every intermediate is
produced in the layout its consumer wants, so there are no on-device transposes.
"""

import sys

if "/opt/trn_rl_repo" not in sys.path:
    sys.path.insert(0, "/opt/trn_rl_repo")

import ml_dtypes
import numpy as np

import concourse.bass as bass
import concourse.mybir as mybir
import concourse.tile as tile
from concourse import bacc, bass_utils
from concourse.bass import ts

# Problem constants (hardcoded per contract)
B, S, D = 2, 2048, 1024
H, DH = 16, 64            # total heads, head dim
HC = 4                    # heads per core
DHC = HC * DH             # 256 projected dims per core
NCORES = 8
P = 128
CH = 512                  # query-chunk for attention / projection sub-chunk
NCH = S // CH             # 4
TT = S // P               # 16 key tiles
KO = D // P               # 8 contraction tiles for projections

f32 = mybir.dt.float32
bf16 = mybir.dt.bfloat16
EXP = mybir.ActivationFunctionType.Exp

_compiled = None          # cached nc across calls
last_results = None       # BassKernelResults of the most recent run (for profiling)


def _build():
    nc = bacc.Bacc("TRN2", target_bir_lowering=False, debug=False)

    # Per-core DRAM parameters. Activations pre-transposed AND pre-chunked on
    # host: x[c, p, ko, s] = x^T[ko*128+p, c*512+s], so each chunk is 8KB
    # contiguous per partition (fat DMA descriptors).
    qT = nc.dram_tensor("qT", [NCH, P, KO, CH], bf16, kind="ExternalInput")
    kT = nc.dram_tensor("kT", [NCH, P, KO, CH], bf16, kind="ExternalInput")
    vT = nc.dram_tensor("vT", [NCH, P, KO, CH], bf16, kind="ExternalInput")
    # Weights pre-arranged to [P, KO, .] on host (4KB/partition descriptors).
    wq = nc.dram_tensor("wq", [P, KO, DHC], bf16, kind="ExternalInput")
    wk = nc.dram_tensor("wk", [P, KO, DHC], bf16, kind="ExternalInput")
    wv = nc.dram_tensor("wv", [P, KO, DHC], bf16, kind="ExternalInput")
    wo = nc.dram_tensor("wo", [P, DHC // P, D], bf16, kind="ExternalInput")
    bq = nc.dram_tensor("bq", [DHC], f32, kind="ExternalInput")
    bk = nc.dram_tensor("bk", [DHC], f32, kind="ExternalInput")
    bv = nc.dram_tensor("bv", [DHC], f32, kind="ExternalInput")
    out = nc.dram_tensor("out", [S, D], bf16, kind="ExternalOutput")

    with tile.TileContext(nc) as tc:
        with (
            tc.tile_pool(name="weights", bufs=1) as wpool,
            tc.tile_pool(name="acts", bufs=1) as apool,
            tc.tile_pool(name="xin", bufs=5) as xpool,
            tc.tile_pool(name="pt", bufs=6) as ptpool,
            tc.tile_pool(name="small", bufs=2) as spool,
            tc.tile_pool(name="outs", bufs=2) as opool,
            tc.tile_pool(name="io_ps", bufs=2, space="PSUM") as io_ps,
            tc.tile_pool(name="l_ps", bufs=3, space="PSUM") as l_ps,
        ):
            # ---- persistent SBUF tiles ----
            wq_sb = wpool.tile([P, KO, DHC], bf16, tag="wq")
            wk_sb = wpool.tile([P, KO, DHC], bf16, tag="wk")
            wv_sb = wpool.tile([P, KO, DHC], bf16, tag="wv")
            wo_sb = wpool.tile([P, DHC // P, D], bf16, tag="wo")
            bq_sb = wpool.tile([P, 2], f32, tag="bq")
            bk_sb = wpool.tile([P, 2], f32, tag="bk")
            bv_row = wpool.tile([P, DHC], f32, tag="bv_row")
            bv_bc = wpool.tile([P, DHC], f32, tag="bv_bc")
            bv_heads = bv_bc[:, :].rearrange("p (h c) -> p h c", c=DH)
            warm = wpool.tile([P, 2], f32, tag="warm")

            # q^T/k^T: [P, m, S] where projected dim r lives at (r % 128, r // 128)
            q_sb = apool.tile([P, 2, S], bf16, tag="q")
            k_sb = apool.tile([P, 2, S], bf16, tag="k")
            # v natural + ones column per head (65-strided), padded so every
            # head's stationary slice can be 128 columns wide.
            VW = HC * (DH + 1)
            v_sb = apool.tile([P, TT, VW + P - (DH + 1)], bf16, tag="v")
            v_heads = v_sb[:, :, 0:VW].rearrange("p tt (h c) -> p tt h c", c=DH + 1)
            attn_sb = apool.tile([P, 2, S], bf16, tag="attn")
            ones_f32 = wpool.tile([P, TT, HC], f32, tag="ones")

            # ---- cheap setup first: the warmup EXP's const-bias DMA must hit
            # the SP queue before the big input transfers, so the ACT table
            # load happens at t~1us instead of behind 11MB of DMA.
            nc.vector.memset(ones_f32, 1.0)
            nc.scalar.activation(out=warm[0:1, :], in_=ones_f32[0:1, 0, 0:2], func=EXP)
            nc.vector.tensor_copy(out=v_heads[:, :, :, DH], in_=ones_f32)
            nc.vector.memset(v_sb[:, :, VW:], 0.0)

            # ---- DMA emission in consumption-priority order. Each dma_start
            # lands on one ~26GB/s hardware queue, so latency-critical
            # transfers are split into pieces to engage several queues.
            # Each dma_start is a ~600ns SP-sequencer instruction and lands
            # on one ~26GB/s HW queue; the critical wave (wk, k/q chunk 0) is
            # split into 128KB pieces, first halves ahead of second halves.
            def dma_range(dst, src, k0, k1, step, eng=None):
                eng = eng or nc.sync
                for i in range(k0, k1, step):
                    ksl = slice(i, i + step)
                    eng.dma_start(out=dst[:, ksl], in_=src[:, ksl])

            xk = {}
            xq = {}
            xv = {}
            xk[0] = xpool.tile([P, KO, CH], bf16, tag="x", name="xk0")
            xq[0] = xpool.tile([P, KO, CH], bf16, tag="x", name="xq0")
            dma_range(wk_sb, wk.ap(), 0, 8, 2)
            dma_range(xk[0], kT.ap()[0], 0, 4, 1)
            dma_range(wq_sb, wq.ap(), 0, 8, 2)
            dma_range(xq[0], qT.ap()[0], 0, 4, 1)
            dma_range(xk[0], kT.ap()[0], 4, 8, 1)
            dma_range(xq[0], qT.ap()[0], 4, 8, 1)
            nc.sync.dma_start(out=bk_sb, in_=bk.ap().rearrange("(mo p) -> p mo", p=P))
            nc.sync.dma_start(out=bq_sb, in_=bq.ap().rearrange("(mo p) -> p mo", p=P))
            nc.sync.dma_start(out=bv_row[0:1, :], in_=bv.ap().rearrange("(a d) -> a d", a=1))
            for c in range(1, NCH):
                xk[c] = xpool.tile([P, KO, CH], bf16, tag="x", name=f"xk{c}")
                dma_range(xk[c], kT.ap()[c], 0, 8, 4)
            xq[1] = xpool.tile([P, KO, CH], bf16, tag="x", name="xq1")
            dma_range(xq[1], qT.ap()[1], 0, 8, 4)
            xq[2] = xpool.tile([P, KO, CH], bf16, tag="x", name="xq2")
            dma_range(xq[2], qT.ap()[2], 0, 8, 4)
            xq[3] = xpool.tile([P, KO, CH], bf16, tag="x", name="xq3")
            dma_range(xq[3], qT.ap()[3], 0, 8, 4)
            dma_range(wv_sb, wv.ap(), 0, 8, 4)
            for c in range(NCH):
                xv[c] = xpool.tile([P, KO, CH], bf16, tag="x", name=f"xv{c}")
                dma_range(xv[c], vT.ap()[c], 0, 8, 4)
            nc.sync.dma_start(out=wo_sb[:, 0], in_=wo.ap()[:, 0])
            nc.sync.dma_start(out=wo_sb[:, 1], in_=wo.ap()[:, 1])

            nc.gpsimd.partition_broadcast(bv_bc, bv_row[0:1, :])

            # ---- emission helpers ----
            # Projections are emitted through a FIFO of small "filler pieces"
            # (2-4 matmuls each) interleaved between QK tiles: the exp stream
            # paces QK via the l_ps pool at ~1.06us/tile while a QK tile is
            # only ~0.43us of PE work, so each tile slot calls fill(2) to top
            # the PE up and keep its clock from dropping. A projection's PSUM
            # accumulation stays open across pieces (different banks than QK).
            fill_q = []
            _fseq = [0]

            def _fname():
                _fseq[0] += 1
                return f"fps{_fseq[0]}"

            def add_kq(c, w_sb, b_sb, xd, dst, m):
                sl = slice(c * CH, (c + 1) * CH)
                state = {}

                def piece(k0):
                    def f():
                        if "ps" not in state:
                            state["ps"] = io_ps.tile([P, CH], f32, tag="ps",
                                                     name=_fname())
                        ps = state["ps"]
                        for ko in range(k0, k0 + 2):
                            nc.tensor.matmul(ps, w_sb[:, ko, ts(m, P)],
                                             xd[:, ko, :],
                                             start=(ko == 0), stop=(ko == KO - 1))
                        if k0 + 2 == KO:
                            nc.vector.tensor_scalar_add(
                                out=dst[:, m, sl], in0=ps,
                                scalar1=b_sb[:, m : m + 1])
                    return f

                for k0 in range(0, KO, 2):
                    fill_q.append(piece(k0))

            def add_kp(c, m):
                add_kq(c, wk_sb, bk_sb, xk[c], k_sb, m)

            def add_qp(c, m):
                add_kq(c, wq_sb, bq_sb, xq[c], q_sb, m)

            def add_vp(c, th):
                tt = (c * CH) // P + th
                state = {}

                def piece(k0):
                    def f():
                        if "ps" not in state:
                            state["ps"] = io_ps.tile([P, CH], f32, tag="ps",
                                                     name=_fname())
                        ps = state["ps"]
                        for ko in range(k0, k0 + 4):
                            nc.tensor.matmul(ps[:, 0:DHC],
                                             xv[c][:, ko, ts(th, P)],
                                             wv_sb[:, ko, :],
                                             start=(ko == 0), stop=(ko == KO - 1))
                        if k0 + 4 == KO:
                            nc.vector.tensor_add(
                                out=v_heads[:, tt, :, 0:DH],
                                in0=ps[:, 0:DHC].rearrange("p (h c) -> p h c",
                                                           c=DH),
                                in1=bv_heads,
                            )
                    return f

                for k0 in range(0, KO, 4):
                    fill_q.append(piece(k0))

            def fill(n):
                for _ in range(n):
                    if fill_q:
                        fill_q.pop(0)()

            pts = {}  # (c, h) -> exp'd logits [P, TT, CH], keys on partitions

            def qk_head(c, h, tbs):
                """QK + exp for query-chunk c, head h, tb pairs in tbs."""
                csl = slice(c * CH, (c + 1) * CH)
                if (c, h) not in pts:
                    pts[(c, h)] = ptpool.tile([P, TT, CH], bf16, tag="pt",
                                              name=f"pt_c{c}_h{h}")
                base = DH * (h % 2)
                m = h // 2
                for tb in tbs:
                    ps = l_ps.tile([P, 2, CH], f32, tag="l")
                    for j in range(2):
                        tt = 2 * tb + j
                        nc.tensor.matmul(
                            ps[:, j, :],
                            k_sb[base : base + DH, m, ts(tt, P)],
                            q_sb[base : base + DH, m, csl],
                            start=True, stop=True,
                        )
                    nc.scalar.activation(out=pts[(c, h)][:, 2 * tb : 2 * tb + 2, :],
                                         in_=ps, func=EXP)

            def pv_head(c, h):
                """PV + normalize for query-chunk c, head h."""
                csl = slice(c * CH, (c + 1) * CH)
                base = DH * (h % 2)
                m = h // 2
                po = io_ps.tile([P, CH], f32, tag="ps")
                for tt in range(TT):
                    nc.tensor.matmul(
                        po[0 : DH + 1, :],
                        v_heads[:, tt, h, :],
                        pts[(c, h)][:, tt, :],
                        start=(tt == 0), stop=(tt == TT - 1),
                    )
                # approx reciprocal must read SBUF at base partition 0 (the
                # custom-DVE op mis-addresses PSUM rows at nonzero bases)
                rin = spool.tile([P, CH], f32, tag="rin")
                nc.vector.tensor_copy(out=rin[0:1, :], in_=po[DH : DH + 1, :])
                rec = spool.tile([P, CH], f32, tag="rec")
                nc.vector.reciprocal_approx_fast(out=rec[0:1, :], in_=rin[0:1, :])
                bc = spool.tile([P, CH], f32, tag="bc")
                nc.gpsimd.partition_broadcast(bc[0:DH, :], rec[0:1, :])
                nc.vector.tensor_mul(
                    out=attn_sb[base : base + DH, m, csl],
                    in0=po[0:DH, :], in1=bc[0:DH, :],
                )

            def out_proj(c):
                # Last chunk: evacuate PSUM on the (by then idle) ACT engine so
                # the PE doesn't stall on DVE evacs in the tail; split each
                # store across two DMA queues to shrink the drain.
                last = c == NCH - 1
                for th in range(CH // P):
                    st = (c * CH) // P + th
                    for n in range(2):
                        pw = io_ps.tile([P, CH], f32, tag="ps")
                        for ko in range(2):
                            nc.tensor.matmul(pw, attn_sb[:, ko, ts(st, P)],
                                             wo_sb[:, ko, ts(n, 512)],
                                             start=(ko == 0), stop=(ko == 1))
                        ot = opool.tile([P, CH], bf16, tag="ot")
                        if last and n == 0:
                            nc.scalar.copy(out=ot, in_=pw)
                        else:
                            nc.vector.tensor_copy(out=ot, in_=pw)
                        for half in range(2):
                            hs = slice(half * 256, (half + 1) * 256)
                            nc.sync.dma_start(
                                out=out.ap()[ts(st, P), n * 512 + half * 256 :
                                             n * 512 + (half + 1) * 256],
                                in_=ot[:, hs])

            # ---- phase A: chunk-0 QK staged by key chunk at l_ps-tile
            # granularity, with projection work as filler between QK tiles so
            # the in-order PE queue never runs dry while the exp stream paces.
            # Stage kc's QK tiles read K tiles of k-chunk kc, so kp(kc) must
            # be fully emitted before that stage; Vp/Qp filler is ordered to
            # match DMA arrival.
            def qk1(h, tb):
                qk_head(0, h, (tb,))

            def kq_direct(c, w_sb, b_sb, xd, dst, m):
                sl = slice(c * CH, (c + 1) * CH)
                ps = io_ps.tile([P, CH], f32, tag="ps")
                for ko in range(KO):
                    nc.tensor.matmul(ps, w_sb[:, ko, ts(m, P)], xd[:, ko, :],
                                     start=(ko == 0), stop=(ko == KO - 1))
                nc.vector.tensor_scalar_add(out=dst[:, m, sl], in0=ps,
                                            scalar1=b_sb[:, m : m + 1])

            # critical path straight-line: K/Q chunk-0 m0 projections
            kq_direct(0, wk_sb, bk_sb, xk[0], k_sb, 0)
            kq_direct(0, wq_sb, bq_sb, xq[0], q_sb, 0)
            # filler FIFO in dependency/DMA-arrival order
            add_kp(0, 1); add_qp(0, 1)
            add_kp(1, 0); add_kp(1, 1)
            add_qp(1, 0); add_qp(1, 1)
            add_kp(2, 0); add_kp(2, 1)
            add_kp(3, 0); add_kp(3, 1)
            add_qp(2, 0); add_qp(2, 1)
            add_qp(3, 0); add_qp(3, 1)
            for c in range(NCH):
                for th in range(CH // P):
                    add_vp(c, th)
            # stage 0: m0 heads first (their inputs are ready), m1 heads after
            # the kp/qp(0,1) fillers have drained
            for tb in (0, 1):
                qk1(0, tb); fill(2)
                qk1(1, tb); fill(2)
            for tb in (0, 1):
                qk1(2, tb); fill(2)
                qk1(3, tb); fill(2)
            # stages 1-3
            for tb in range(2, 8):
                for h in range(HC):
                    qk1(h, tb); fill(2)
            for h in (0, 1):                   # 1-chunk QK lookahead
                for tb in range(8):
                    qk_head(1, h, (tb,))
                    fill(2)
            while fill_q:                      # drain: all V must be resident
                fill(1)                        # before the first PV below

            # ---- steady state ----
            # Per chunk c: blocks of [PV(c,h) + normalize, next chunk's QK
            # tiles, out-proj groups of chunk c-1]. The QK tiles are paced by
            # the exp stream via the l_ps pool; PV and out groups sit between
            # them so the PE's 4-deep lookahead always has runnable work and
            # the clock never drops. out(c3) runs as a tail burst with ACT
            # evacuation (ACT is idle by then).
            def out_group(c, g):
                st = (c * CH) // P + g // 2
                n = g % 2
                last = c == NCH - 1
                pw = io_ps.tile([P, CH], f32, tag="ps")
                for ko in range(2):
                    nc.tensor.matmul(pw, attn_sb[:, ko, ts(st, P)],
                                     wo_sb[:, ko, ts(n, 512)],
                                     start=(ko == 0), stop=(ko == 1))
                ot = opool.tile([P, CH], bf16, tag="ot")
                if last:
                    nc.scalar.copy(out=ot, in_=pw)
                else:
                    nc.vector.tensor_copy(out=ot, in_=pw)
                for half in range(2):
                    nc.sync.dma_start(
                        out=out.ap()[ts(st, P), n * 512 + half * 256 :
                                     n * 512 + (half + 1) * 256],
                        in_=ot[:, half * 256 : (half + 1) * 256])

            for c in range(NCH):
                for h in range(HC):
                    pv_head(c, h)
                    qh = h + 2 if c == 0 else h  # c0 continues at (c1, h2)
                    tiles = list(range(8)) if c + 1 < NCH and qh < HC else []
                    for i, tb in enumerate(tiles):
                        qk_head(c + 1, qh, (tb,))
                        if c > 0 and i in (1, 4):
                            out_group(c - 1, 2 * h + (0 if i == 1 else 1))
                    if not tiles and c > 0:
                        out_group(c - 1, 2 * h)
                        out_group(c - 1, 2 * h + 1)
            out_proj(NCH - 1)

    nc.finalize()
    return nc


def kernel(**inputs):
    global _compiled, last_results
    if _compiled is None:
        _compiled = _build()
    nc = _compiled

    query = np.asarray(inputs["query"], np.float32)
    key = np.asarray(inputs["key"], np.float32)
    value = np.asarray(inputs["value"], np.float32)
    Wq = np.asarray(inputs["Wq"], np.float32)
    Wk = np.asarray(inputs["Wk"], np.float32)
    Wv = np.asarray(inputs["Wv"], np.float32)
    Wo = np.asarray(inputs["Wo"], np.float32)
    bq_f = np.asarray(inputs["bq"], np.float32)
    bk_f = np.asarray(inputs["bk"], np.float32)
    bv_f = np.asarray(inputs["bv"], np.float32)
    bo_f = np.asarray(inputs["bo"], np.float32)

    bf = ml_dtypes.bfloat16
    scale = 1.0 / np.sqrt(np.float32(DH))

    def chunked(x):  # [S, D] -> [NCH, P, KO, CH] with x^T chunk-contiguous
        xt = np.ascontiguousarray(x.T)                       # [D, S]
        return np.ascontiguousarray(
            xt.reshape(KO, P, NCH, CH).transpose(2, 1, 0, 3)
        ).astype(bf)

    def wlayout(w):  # [D, DHC] -> [P, KO, DHC]
        return np.ascontiguousarray(w.reshape(KO, P, DHC).transpose(1, 0, 2)).astype(bf)

    qTc = [chunked(query[b]) for b in range(B)]
    kTc = [chunked(key[b]) for b in range(B)]
    vTc = [chunked(value[b]) for b in range(B)]

    in_maps = []
    for c in range(NCORES):
        b = c // 4
        sh = c % 4
        sl = slice(DHC * sh, DHC * (sh + 1))
        in_maps.append({
            "qT": qTc[b], "kT": kTc[b], "vT": vTc[b],
            "wq": wlayout(Wq[:, sl] * scale),
            "wk": wlayout(Wk[:, sl]),
            "wv": wlayout(Wv[:, sl]),
            "wo": np.ascontiguousarray(
                Wo[sl, :].reshape(DHC // P, P, D).transpose(1, 0, 2)
            ).astype(bf),
            "bq": np.ascontiguousarray(bq_f[sl]) * scale,
            "bk": np.ascontiguousarray(bk_f[sl]),
            "bv": np.ascontiguousarray(bv_f[sl]),
        })

    res = bass_utils.run_bass_kernel_spmd(nc, in_maps, core_ids=list(range(NCORES)))
    last_results = res

    final = np.empty((B, S, D), np.float32)
    for b in range(B):
        acc = res.results[4 * b]["out"].astype(np.float32)
        for sh in range(1, 4):
            acc = acc + res.results[4 * b + sh]["out"].astype(np.float32)
        final[b] = acc + bo_f
    return final


# revision 27
# speedup vs baseline: 1.0524x; 1.0043x over previous
"""Multi-head attention (B=2, S=2048, D=1024, H=16, dh=64) on 8 TRN2 NeuronCores.

Sharding: data-parallel over batch (2) x tensor-parallel over heads (4 per core).
Core c handles batch c//4 and heads [4*(c%4), 4*(c%4)+4). Each core computes a
partial output (its heads' contribution through Wo); the host sums the 4 partials
per batch and adds bo (the unshard step for a sum-sharded tensor).

The schedule is built around the Activation engine: exp over the full S x S x 4
logits is ~137us of ACT time, more than the PE's ~109us of attention matmuls, so
ACT must start early and never starve. To that end:
  - Host supplies activations in chunk-contiguous layout ([NCH, P, KO, CH], 8KB
    per-partition descriptors) and DMAs are emitted in consumption order, so the
    K projection starts ~5us in.
  - QK for query-chunk 0 is staged per KEY chunk: logits for key tiles of chunk
    kc are computed (and exp'd) right after the K projection of chunk kc, so the
    first EXP issues at ~12us instead of after the full K projection.
  - V/Q projections, the next chunk's QK, and the output projection of the
    previous chunk are interleaved into the attention loop, filling the PE while
    ACT paces, and eliminating the output-projection tail.
  - Softmax normalization uses reciprocal_approx_fast (single DVE op, ~5x faster
    than the exact iterative divide; denominators are sums of positive exps so
    the approx domain is safe) and multiplies straight out of PSUM.

Matmuls run in bf16 with f32 PSUM accumulation. The PV matmul carries an extra
ones-column in the stationary operand so the softmax denominator falls out of
the same accumulation for free; bv is pre-added to V (P @ (V + 1*bv) = PV +
denom*bv, so the post-divide result already includes bv). Every intermediate is
produced in the layout its consumer wants, so there are no on-device transposes.
"""

import sys

if "/opt/trn_rl_repo" not in sys.path:
    sys.path.insert(0, "/opt/trn_rl_repo")

import ml_dtypes
import numpy as np

import concourse.bass as bass
import concourse.mybir as mybir
import concourse.tile as tile
from concourse import bacc, bass_utils
from concourse.bass import ts

# Problem constants (hardcoded per contract)
B, S, D = 2, 2048, 1024
H, DH = 16, 64            # total heads, head dim
HC = 4                    # heads per core
DHC = HC * DH             # 256 projected dims per core
NCORES = 8
P = 128
CH = 512                  # query-chunk for attention / projection sub-chunk
NCH = S // CH             # 4
TT = S // P               # 16 key tiles
KO = D // P               # 8 contraction tiles for projections

f32 = mybir.dt.float32
bf16 = mybir.dt.bfloat16
EXP = mybir.ActivationFunctionType.Exp

_compiled = None          # cached nc across calls
last_results = None       # BassKernelResults of the most recent run (for profiling)


def _build():
    nc = bacc.Bacc("TRN2", target_bir_lowering=False, debug=False)

    # Per-core DRAM parameters. Activations pre-transposed AND pre-chunked on
    # host: x[c, p, ko, s] = x^T[ko*128+p, c*512+s], so each chunk is 8KB
    # contiguous per partition (fat DMA descriptors).
    qT = nc.dram_tensor("qT", [NCH, P, KO, CH], bf16, kind="ExternalInput")
    kT = nc.dram_tensor("kT", [NCH, P, KO, CH], bf16, kind="ExternalInput")
    vT = nc.dram_tensor("vT", [NCH, P, KO, CH], bf16, kind="ExternalInput")
    # Weights pre-arranged to [P, KO, .] on host (4KB/partition descriptors).
    wq = nc.dram_tensor("wq", [P, KO, DHC], bf16, kind="ExternalInput")
    wk = nc.dram_tensor("wk", [P, KO, DHC], bf16, kind="ExternalInput")
    wv = nc.dram_tensor("wv", [P, KO, DHC], bf16, kind="ExternalInput")
    wo = nc.dram_tensor("wo", [P, DHC // P, D], bf16, kind="ExternalInput")
    bq = nc.dram_tensor("bq", [DHC], f32, kind="ExternalInput")
    bk = nc.dram_tensor("bk", [DHC], f32, kind="ExternalInput")
    bv = nc.dram_tensor("bv", [DHC], f32, kind="ExternalInput")
    out = nc.dram_tensor("out", [S, D], bf16, kind="ExternalOutput")

    with tile.TileContext(nc) as tc:
        with (
            tc.tile_pool(name="weights", bufs=1) as wpool,
            tc.tile_pool(name="acts", bufs=1) as apool,
            tc.tile_pool(name="xin", bufs=5) as xpool,
            tc.tile_pool(name="pt", bufs=6) as ptpool,
            tc.tile_pool(name="small", bufs=2) as spool,
            tc.tile_pool(name="outs", bufs=2) as opool,
            tc.tile_pool(name="io_ps", bufs=2, space="PSUM") as io_ps,
            tc.tile_pool(name="l_ps", bufs=3, space="PSUM") as l_ps,
        ):
            # ---- persistent SBUF tiles ----
            wq_sb = wpool.tile([P, KO, DHC], bf16, tag="wq")
            wk_sb = wpool.tile([P, KO, DHC], bf16, tag="wk")
            wv_sb = wpool.tile([P, KO, DHC], bf16, tag="wv")
            wo_sb = wpool.tile([P, DHC // P, D], bf16, tag="wo")
            bq_sb = wpool.tile([P, 2], f32, tag="bq")
            bk_sb = wpool.tile([P, 2], f32, tag="bk")
            bv_row = wpool.tile([P, DHC], f32, tag="bv_row")
            bv_bc = wpool.tile([P, DHC], f32, tag="bv_bc")
            bv_heads = bv_bc[:, :].rearrange("p (h c) -> p h c", c=DH)
            warm = wpool.tile([P, 2], f32, tag="warm")

            # q^T/k^T: [P, m, S] where projected dim r lives at (r % 128, r // 128)
            q_sb = apool.tile([P, 2, S], bf16, tag="q")
            k_sb = apool.tile([P, 2, S], bf16, tag="k")
            # v natural + ones column per head (65-strided), padded so every
            # head's stationary slice can be 128 columns wide.
            VW = HC * (DH + 1)
            v_sb = apool.tile([P, TT, VW + P - (DH + 1)], bf16, tag="v")
            v_heads = v_sb[:, :, 0:VW].rearrange("p tt (h c) -> p tt h c", c=DH + 1)
            attn_sb = apool.tile([P, 2, S], bf16, tag="attn")
            ones_f32 = wpool.tile([P, TT, HC], f32, tag="ones")

            # ---- cheap setup first: the warmup EXP's const-bias DMA must hit
            # the SP queue before the big input transfers, so the ACT table
            # load happens at t~1us instead of behind 11MB of DMA.
            nc.vector.memset(ones_f32, 1.0)
            nc.scalar.activation(out=warm[0:1, :], in_=ones_f32[0:1, 0, 0:2], func=EXP)
            nc.vector.tensor_copy(out=v_heads[:, :, :, DH], in_=ones_f32)
            nc.vector.memset(v_sb[:, :, VW:], 0.0)

            # ---- DMA emission in consumption-priority order. Each dma_start
            # lands on one ~26GB/s hardware queue, so latency-critical
            # transfers are split into pieces to engage several queues.
            # Each dma_start is a ~600ns SP-sequencer instruction and lands
            # on one ~26GB/s HW queue; the critical wave (wk, k/q chunk 0) is
            # split into 128KB pieces, first halves ahead of second halves.
            def dma_range(dst, src, k0, k1, step, eng=None):
                eng = eng or nc.sync
                for i in range(k0, k1, step):
                    ksl = slice(i, i + step)
                    eng.dma_start(out=dst[:, ksl], in_=src[:, ksl])

            xk = {}
            xq = {}
            xv = {}
            xk[0] = xpool.tile([P, KO, CH], bf16, tag="x", name="xk0")
            xq[0] = xpool.tile([P, KO, CH], bf16, tag="x", name="xq0")
            dma_range(wk_sb, wk.ap(), 0, 8, 2)
            dma_range(xk[0], kT.ap()[0], 0, 4, 1)
            dma_range(wq_sb, wq.ap(), 0, 8, 2)
            dma_range(xq[0], qT.ap()[0], 0, 4, 1)
            dma_range(xk[0], kT.ap()[0], 4, 8, 1)
            dma_range(xq[0], qT.ap()[0], 4, 8, 1)
            nc.sync.dma_start(out=bk_sb, in_=bk.ap().rearrange("(mo p) -> p mo", p=P))
            nc.sync.dma_start(out=bq_sb, in_=bq.ap().rearrange("(mo p) -> p mo", p=P))
            nc.sync.dma_start(out=bv_row[0:1, :], in_=bv.ap().rearrange("(a d) -> a d", a=1))
            for c in range(1, NCH):
                xk[c] = xpool.tile([P, KO, CH], bf16, tag="x", name=f"xk{c}")
                dma_range(xk[c], kT.ap()[c], 0, 8, 4)
            xq[1] = xpool.tile([P, KO, CH], bf16, tag="x", name="xq1")
            dma_range(xq[1], qT.ap()[1], 0, 8, 4)
            xq[2] = xpool.tile([P, KO, CH], bf16, tag="x", name="xq2")
            dma_range(xq[2], qT.ap()[2], 0, 8, 4)
            xq[3] = xpool.tile([P, KO, CH], bf16, tag="x", name="xq3")
            dma_range(xq[3], qT.ap()[3], 0, 8, 4)
            dma_range(wv_sb, wv.ap(), 0, 8, 4)
            for c in range(NCH):
                xv[c] = xpool.tile([P, KO, CH], bf16, tag="x", name=f"xv{c}")
                dma_range(xv[c], vT.ap()[c], 0, 8, 4)
            nc.sync.dma_start(out=wo_sb[:, 0], in_=wo.ap()[:, 0])
            nc.sync.dma_start(out=wo_sb[:, 1], in_=wo.ap()[:, 1])

            nc.gpsimd.partition_broadcast(bv_bc, bv_row[0:1, :])

            # ---- emission helpers ----
            # Projections are emitted through a FIFO of small "filler pieces"
            # (2-4 matmuls each) interleaved between QK tiles: the exp stream
            # paces QK via the l_ps pool at ~1.06us/tile while a QK tile is
            # only ~0.43us of PE work, so each tile slot calls fill(2) to top
            # the PE up and keep its clock from dropping. A projection's PSUM
            # accumulation stays open across pieces (different banks than QK).
            fill_q = []
            _fseq = [0]

            def _fname():
                _fseq[0] += 1
                return f"fps{_fseq[0]}"

            def add_kq(c, w_sb, b_sb, xd, dst, m):
                sl = slice(c * CH, (c + 1) * CH)
                state = {}

                def piece(k0):
                    def f():
                        if "ps" not in state:
                            state["ps"] = io_ps.tile([P, CH], f32, tag="ps",
                                                     name=_fname())
                        ps = state["ps"]
                        for ko in range(k0, k0 + 2):
                            nc.tensor.matmul(ps, w_sb[:, ko, ts(m, P)],
                                             xd[:, ko, :],
                                             start=(ko == 0), stop=(ko == KO - 1))
                        if k0 + 2 == KO:
                            nc.vector.tensor_scalar_add(
                                out=dst[:, m, sl], in0=ps,
                                scalar1=b_sb[:, m : m + 1])
                    return f

                for k0 in range(0, KO, 2):
                    fill_q.append(piece(k0))

            def add_kp(c, m):
                add_kq(c, wk_sb, bk_sb, xk[c], k_sb, m)

            def add_qp(c, m):
                add_kq(c, wq_sb, bq_sb, xq[c], q_sb, m)

            def add_vp(c, th):
                tt = (c * CH) // P + th
                state = {}

                def piece(k0):
                    def f():
                        if "ps" not in state:
                            state["ps"] = io_ps.tile([P, CH], f32, tag="ps",
                                                     name=_fname())
                        ps = state["ps"]
                        for ko in range(k0, k0 + 4):
                            nc.tensor.matmul(ps[:, 0:DHC],
                                             xv[c][:, ko, ts(th, P)],
                                             wv_sb[:, ko, :],
                                             start=(ko == 0), stop=(ko == KO - 1))
                        if k0 + 4 == KO:
                            nc.vector.tensor_add(
                                out=v_heads[:, tt, :, 0:DH],
                                in0=ps[:, 0:DHC].rearrange("p (h c) -> p h c",
                                                           c=DH),
                                in1=bv_heads,
                            )
                    return f

                for k0 in range(0, KO, 4):
                    fill_q.append(piece(k0))

            def fill(n):
                for _ in range(n):
                    if fill_q:
                        fill_q.pop(0)()

            pts = {}  # (c, h) -> exp'd logits [P, TT, CH], keys on partitions

            def qk_head(c, h, tbs):
                """QK + exp for query-chunk c, head h, tb pairs in tbs."""
                csl = slice(c * CH, (c + 1) * CH)
                if (c, h) not in pts:
                    pts[(c, h)] = ptpool.tile([P, TT, CH], bf16, tag="pt",
                                              name=f"pt_c{c}_h{h}")
                base = DH * (h % 2)
                m = h // 2
                for tb in tbs:
                    ps = l_ps.tile([P, 2, CH], f32, tag="l")
                    for j in range(2):
                        tt = 2 * tb + j
                        nc.tensor.matmul(
                            ps[:, j, :],
                            k_sb[base : base + DH, m, ts(tt, P)],
                            q_sb[base : base + DH, m, csl],
                            start=True, stop=True,
                        )
                    nc.scalar.activation(out=pts[(c, h)][:, 2 * tb : 2 * tb + 2, :],
                                         in_=ps, func=EXP)

            def pv_head(c, h):
                """PV + normalize for query-chunk c, head h."""
                csl = slice(c * CH, (c + 1) * CH)
                base = DH * (h % 2)
                m = h // 2
                po = io_ps.tile([P, CH], f32, tag="ps")
                for tt in range(TT):
                    nc.tensor.matmul(
                        po[0 : DH + 1, :],
                        v_heads[:, tt, h, :],
                        pts[(c, h)][:, tt, :],
                        start=(tt == 0), stop=(tt == TT - 1),
                    )
                # approx reciprocal must read SBUF at base partition 0 (the
                # custom-DVE op mis-addresses PSUM rows at nonzero bases, and
                # PSUM reads of >32 partitions must start at partition 0)
                rin = spool.tile([P, CH], f32, tag="rin")
                nc.vector.tensor_copy(out=rin[0:1, :], in_=po[DH : DH + 1, :])
                rec = spool.tile([P, CH], f32, tag="rec")
                nc.vector.reciprocal_approx_fast(out=rec[0:1, :], in_=rin[0:1, :])
                bc = spool.tile([P, CH], f32, tag="bc")
                nc.gpsimd.partition_broadcast(bc[0:DH, :], rec[0:1, :])
                nc.vector.tensor_mul(
                    out=attn_sb[base : base + DH, m, csl],
                    in0=po[0:DH, :], in1=bc[0:DH, :],
                )

            def out_proj(c):
                # Last chunk: evacuate PSUM on the (by then idle) ACT engine so
                # the PE doesn't stall on DVE evacs in the tail; split each
                # store across two DMA queues to shrink the drain.
                last = c == NCH - 1
                for th in range(CH // P):
                    st = (c * CH) // P + th
                    for n in range(2):
                        pw = io_ps.tile([P, CH], f32, tag="ps")
                        for ko in range(2):
                            nc.tensor.matmul(pw, attn_sb[:, ko, ts(st, P)],
                                             wo_sb[:, ko, ts(n, 512)],
                                             start=(ko == 0), stop=(ko == 1))
                        ot = opool.tile([P, CH], bf16, tag="ot")
                        if last and n == 0:
                            nc.scalar.copy(out=ot, in_=pw)
                        else:
                            nc.vector.tensor_copy(out=ot, in_=pw)
                        for half in range(2):
                            hs = slice(half * 256, (half + 1) * 256)
                            nc.sync.dma_start(
                                out=out.ap()[ts(st, P), n * 512 + half * 256 :
                                             n * 512 + (half + 1) * 256],
                                in_=ot[:, hs])

            # ---- phase A: chunk-0 QK staged by key chunk at l_ps-tile
            # granularity, with projection work as filler between QK tiles so
            # the in-order PE queue never runs dry while the exp stream paces.
            # Stage kc's QK tiles read K tiles of k-chunk kc, so kp(kc) must
            # be fully emitted before that stage; Vp/Qp filler is ordered to
            # match DMA arrival.
            def qk1(h, tb):
                qk_head(0, h, (tb,))

            def kq_direct(c, w_sb, b_sb, xd, dst, m):
                sl = slice(c * CH, (c + 1) * CH)
                ps = io_ps.tile([P, CH], f32, tag="ps")
                for ko in range(KO):
                    nc.tensor.matmul(ps, w_sb[:, ko, ts(m, P)], xd[:, ko, :],
                                     start=(ko == 0), stop=(ko == KO - 1))
                nc.vector.tensor_scalar_add(out=dst[:, m, sl], in0=ps,
                                            scalar1=b_sb[:, m : m + 1])

            # critical path straight-line: K/Q chunk-0 m0 projections
            kq_direct(0, wk_sb, bk_sb, xk[0], k_sb, 0)
            kq_direct(0, wq_sb, bq_sb, xq[0], q_sb, 0)
            # filler FIFO in dependency/DMA-arrival order
            add_kp(0, 1); add_qp(0, 1)
            add_kp(1, 0); add_kp(1, 1)
            add_qp(1, 0); add_qp(1, 1)
            add_kp(2, 0); add_kp(2, 1)
            add_kp(3, 0); add_kp(3, 1)
            add_qp(2, 0); add_qp(2, 1)
            add_qp(3, 0); add_qp(3, 1)
            for c in range(NCH):
                for th in range(CH // P):
                    add_vp(c, th)
            # stage 0: m0 heads first (their inputs are ready), m1 heads after
            # the kp/qp(0,1) fillers have drained
            for tb in (0, 1):
                qk1(0, tb); fill(2)
                qk1(1, tb); fill(2)
            for tb in (0, 1):
                qk1(2, tb); fill(2)
                qk1(3, tb); fill(2)
            # stages 1-3
            for tb in range(2, 8):
                for h in range(HC):
                    qk1(h, tb); fill(2)
            for h in (0, 1):                   # 1-chunk QK lookahead
                for tb in range(8):
                    qk_head(1, h, (tb,))
                    fill(2)
            while fill_q:                      # drain: all V must be resident
                fill(1)                        # before the first PV below

            # ---- steady state ----
            # Per chunk c: blocks of [PV(c,h) + normalize, next chunk's QK
            # tiles, out-proj groups of chunk c-1]. The QK tiles are paced by
            # the exp stream via the l_ps pool; PV and out groups sit between
            # them so the PE's 4-deep lookahead always has runnable work and
            # the clock never drops. out(c3) runs as a tail burst with ACT
            # evacuation (ACT is idle by then).
            def out_group(c, g):
                st = (c * CH) // P + g // 2
                n = g % 2
                last = c == NCH - 1
                pw = io_ps.tile([P, CH], f32, tag="ps")
                for ko in range(2):
                    nc.tensor.matmul(pw, attn_sb[:, ko, ts(st, P)],
                                     wo_sb[:, ko, ts(n, 512)],
                                     start=(ko == 0), stop=(ko == 1))
                ot = opool.tile([P, CH], bf16, tag="ot")
                if last:
                    nc.scalar.copy(out=ot, in_=pw)
                else:
                    nc.vector.tensor_copy(out=ot, in_=pw)
                for half in range(2):
                    nc.sync.dma_start(
                        out=out.ap()[ts(st, P), n * 512 + half * 256 :
                                     n * 512 + (half + 1) * 256],
                        in_=ot[:, half * 256 : (half + 1) * 256])

            for c in range(NCH):
                for h in range(HC):
                    pv_head(c, h)
                    qh = h + 2 if c == 0 else h  # c0 continues at (c1, h2)
                    tiles = list(range(8)) if c + 1 < NCH and qh < HC else []
                    for i, tb in enumerate(tiles):
                        qk_head(c + 1, qh, (tb,))
                        if c > 0 and i in (1, 4):
                            out_group(c - 1, 2 * h + (0 if i == 1 else 1))
                    if not tiles and c > 0:
                        out_group(c - 1, 2 * h)
                        out_group(c - 1, 2 * h + 1)
            out_proj(NCH - 1)

    nc.finalize()
    return nc


def kernel(**inputs):
    global _compiled, last_results
    if _compiled is None:
        _compiled = _build()
    nc = _compiled

    query = np.asarray(inputs["query"], np.float32)
    key = np.asarray(inputs["key"], np.float32)
    value = np.asarray(inputs["value"], np.float32)
    Wq = np.asarray(inputs["Wq"], np.float32)
    Wk = np.asarray(inputs["Wk"], np.float32)
    Wv = np.asarray(inputs["Wv"], np.float32)
    Wo = np.asarray(inputs["Wo"], np.float32)
    bq_f = np.asarray(inputs["bq"], np.float32)
    bk_f = np.asarray(inputs["bk"], np.float32)
    bv_f = np.asarray(inputs["bv"], np.float32)
    bo_f = np.asarray(inputs["bo"], np.float32)

    bf = ml_dtypes.bfloat16
    scale = 1.0 / np.sqrt(np.float32(DH))

    def chunked(x):  # [S, D] -> [NCH, P, KO, CH] with x^T chunk-contiguous
        xt = np.ascontiguousarray(x.T)                       # [D, S]
        return np.ascontiguousarray(
            xt.reshape(KO, P, NCH, CH).transpose(2, 1, 0, 3)
        ).astype(bf)

    def wlayout(w):  # [D, DHC] -> [P, KO, DHC]
        return np.ascontiguousarray(w.reshape(KO, P, DHC).transpose(1, 0, 2)).astype(bf)

    qTc = [chunked(query[b]) for b in range(B)]
    kTc = [chunked(key[b]) for b in range(B)]
    vTc = [chunked(value[b]) for b in range(B)]

    in_maps = []
    for c in range(NCORES):
        b = c // 4
        sh = c % 4
        sl = slice(DHC * sh, DHC * (sh + 1))
        in_maps.append({
            "qT": qTc[b], "kT": kTc[b], "vT": vTc[b],
            "wq": wlayout(Wq[:, sl] * scale),
            "wk": wlayout(Wk[:, sl]),
            "wv": wlayout(Wv[:, sl]),
            "wo": np.ascontiguousarray(
                Wo[sl, :].reshape(DHC // P, P, D).transpose(1, 0, 2)
            ).astype(bf),
            "bq": np.ascontiguousarray(bq_f[sl]) * scale,
            "bk": np.ascontiguousarray(bk_f[sl]),
            "bv": np.ascontiguousarray(bv_f[sl]),
        })

    res = bass_utils.run_bass_kernel_spmd(nc, in_maps, core_ids=list(range(NCORES)))
    last_results = res

    final = np.empty((B, S, D), np.float32)
    for b in range(B):
        acc = res.results[4 * b]["out"].astype(np.float32)
        for sh in range(1, 4):
            acc = acc + res.results[4 * b + sh]["out"].astype(np.float32)
        final[b] = acc + bo_f
    return final
